# revision 29
# baseline (speedup 1.0000x reference)
"""Self-contained Trainium2 Bass kernel for deformable conv 2d.

kernel(x, offset, weight) -> out, matching the jax reference:
  x[2,256,64,64] f32, offset[2,18,64,64] f32, weight[256,256,3,3] f32
  -> out[2,256,64,64] f32 (KH=KW=3, stride=1, pad=1, dil=1, DG=1).

Runs SPMD on 8 NeuronCores, data-parallel: core = (batch, spatial quarter).

v3 design (vs the v1 baseline):
  - bilinear sample indices + corner weights precomputed on HOST (numpy);
    the device does only: batched indirect gathers, fused DVE blends,
    PE transposes, and the main matmuls.
  - gathers batched 4 slots (512 descriptors) per instruction to amortize
    the ~1us SWDGE fixed overhead per DMA instruction.
  - blend uses a custom fused DVE op: out = in0*s0 + in1*s1 (per-partition
    scalars), two ops per 128-sample slot (top row pair / bottom row pair).
  - the final top+bottom add either runs as one batched DVE tensor_tensor
    per tap (plan B slots) or rides the PE transpose via PSUM accumulation
    (plan A slots); A_SLOTS tunes the DVE/PE balance.
  - output stored bf16 (halves the output write traffic).
"""

import sys

for _p in ("/opt/trn_rl_repo",):
    if _p not in sys.path:
        sys.path.insert(0, _p)


import numpy as np
import ml_dtypes

import concourse.bass as bass
import concourse.mybir as mybir
import concourse.tile as tile

F32 = mybir.dt.float32
BF16 = mybir.dt.bfloat16
I32 = mybir.dt.int32
I16 = mybir.dt.int16

N, CIN, H, W = 2, 256, 64, 64
COUT = 256
KH = KW = 3
K = KH * KW
S = H * W            # 4096 output positions per batch
SLOC = S // 4        # 1024 per core
TPC = 8              # ts slots per tap (SLOC/128)
NT = K * TPC         # 72 slots of [128 samples]

A_SLOTS = 0          # slots per tap whose T+B add rides the PE transpose
                     # (PE matmul PSUM writes must be is_transpose to hit
                     # sub-bank offsets; accumulate variant pending test)
GATHER_MODE = "tap"   # "slot": indirect_dma_start per slot (128 desc each)
                      # "tap": dma_gather per tap (1024 idx)
                      # "tap2": dma_gather per 2 taps (2048 idx)

AluOp = mybir.AluOpType


# ---------------- custom fused DVE op ----------------

_BLEND2 = []


def _get_blend2():
    """Register (once) and return the fused op out = in0*s0 + in1*s1."""
    if _BLEND2:
        return _BLEND2[0]
    import concourse.dve_ops as dve_ops_mod
    from concourse.dve_ops import DveOp
    from concourse.dve_spec import Spec, Src0, Src1, C0, C1
    from concourse.dve_spec import lower as dve_lower
    from concourse.dve_uop import DveOpSpec

    name = "BLEND2_DEFORM_ANT"
    if name not in dve_ops_mod._SUB_OPCODE_FOR_NAME:
        spec = Spec(body=Src0 * C0 + Src1 * C1)
        row = max(dve_ops_mod._SUB_OPCODE_FOR_NAME.values()) + 1
        assert row < 0x20
        shas = {}
        for ver in ("v3", "v4"):
            dspec = DveOpSpec(
                name=name, opcode=row, uops=dve_lower(spec, ver=ver), rd1_en=True
            )
            shas[ver] = dspec.sha(ver)
        op = DveOp(name, spec, subdim=False, uops_sha=shas)
        dve_ops_mod._SUB_OPCODE_FOR_NAME[name] = row
        dve_ops_mod.OPS.append(op)
        dve_ops_mod.CUSTOM_DVE_SPECS[name] = spec
    _BLEND2.append(
        next(op for op in dve_ops_mod.OPS if op.name == name)
    )
    return _BLEND2[0]


def build_core_kernel(nc, tc, outs, ins):
    """Emit the per-core kernel. ins/outs are dicts of DRAM APs."""
    from contextlib import ExitStack

    from concourse.ap import AP

    xi = ins["xi"]          # [4096+64, 512] bf16 y-pair-interleaved image
    wT = ins["wT"]          # [2304, 256] bf16 lhsT
    ridx = ins["ridx"]      # [128, NT] i32 gather row per (partition, slot)
    idx16 = ins["idx16"]    # [128, K*64] i16 wrapped gather indices
    cw = ins["cw"]          # [128, 4*NT] f32 corner weights (T0,B0,T1,B1)
    ident = ins["ident"]    # [128, 128] bf16 identity
    out = outs["out"]       # [128, 2, 1024] bf16

    # overlapped view of xi: row r covers xi rows r and r+1 (the 4 corners)
    xi_ov = AP(xi.tensor, 0, [[2 * CIN, S], [1, 4 * CIN]])

    blend2 = _get_blend2()

    ctx = ExitStack()
    sp = ctx.enter_context(tc.tile_pool(name="static", bufs=1))
    gp = ctx.enter_context(tc.tile_pool(name="gather", bufs=2))
    bp = ctx.enter_context(tc.tile_pool(name="blend", bufs=3))
    rp = ctx.enter_context(tc.tile_pool(name="rhsT", bufs=3))
    pp = ctx.enter_context(tc.tile_pool(name="psum", bufs=1, space="PSUM"))
    tp = ctx.enter_context(tc.tile_pool(name="tpsum", bufs=2, space="PSUM"))

    v = nc.vector

    # ---- static loads (indices first: they gate the gathers) ----
    ridx_s = sp.tile([128, NT], I32, name="ridx_s")
    nc.sync.dma_start(ridx_s[:], ridx)
    idx_s = sp.tile([128, K, 64], I16, name="idx_s")
    nc.sync.dma_start(idx_s[:], idx16.rearrange("p (k c) -> p k c", k=K))
    cw_s = sp.tile([128, 4 * NT], F32, name="cw_s")
    nc.sync.dma_start(cw_s[:], cw)
    wT_s = sp.tile([128, 18, 256], BF16, name="wT_s")
    nc.sync.dma_start(wT_s[:], wT.rearrange("(j p) o -> p j o", p=128))
    id_s = sp.tile([128, 128], BF16, name="id_s")
    nc.sync.dma_start(id_s[:], ident)

    ps = [pp.tile([128, SLOC], F32, name=f"psum{h}") for h in range(2)]
    osb = sp.tile([128, 2, SLOC], BF16, name="osb")

    NB = TPC - A_SLOTS   # plan-B slots per tap (batched DVE add)
    g2_hold = None

    for k in range(K):
        # ---- gathers, one of three mechanisms ----
        if GATHER_MODE == "slot":
            gts = [gp.tile([128, 1024], BF16, name=f"g{ts}") for ts in range(TPC)]
            for ts in range(TPC):
                t = k * TPC + ts
                nc.gpsimd.indirect_dma_start(
                    out=gts[ts][:],
                    out_offset=None,
                    in_=xi,
                    in_offset=bass.IndirectOffsetOnAxis(
                        ap=ridx_s[:, t : t + 1], axis=0
                    ),
                )
        elif GATHER_MODE == "tap":
            g = gp.tile([128, TPC, 1024], BF16, name="g")
            nc.gpsimd.dma_gather(
                out_ap=g[:],
                in_ap=xi_ov,
                idxs_ap=idx_s[:, k],
                num_idxs=SLOC,
                num_idxs_reg=SLOC,
                elem_size=4 * CIN,
                elem_step=2 * CIN,
            )
            gts = [g[:, ts] for ts in range(TPC)]
        elif GATHER_MODE == "tap2":
            if k % 2 == 0:
                ntap = 2 if k + 1 < K else 1
                g2 = gp.tile([128, ntap * TPC, 1024], BF16, name="g2")
                nc.gpsimd.dma_gather(
                    out_ap=g2[:],
                    in_ap=xi_ov,
                    idxs_ap=idx_s[:, k : k + ntap].rearrange("p a c -> p (a c)"),
                    num_idxs=ntap * SLOC,
                    num_idxs_reg=ntap * SLOC,
                    elem_size=4 * CIN,
                    elem_step=2 * CIN,
                )
                g2_hold = g2
            base = (k % 2) * TPC
            gts = [g2_hold[:, base + ts] for ts in range(TPC)]
        else:
            raise ValueError(GATHER_MODE)

        # ---- fused blends: ST = T0*cw0 + T1*cw2, SB = B0*cw1 + B1*cw3 ----
        ST = bp.tile([128, TPC, 256], BF16, name="ST")
        SB = bp.tile([128, TPC, 256], BF16, name="SB")
        for ts in range(TPC):
            t = k * TPC + ts
            g = gts[ts]
            v._custom_dve(
                blend2,
                out=ST[:, ts],
                in0=g[:, 0:256],
                in1=g[:, 512:768],
                s0=cw_s[:, 0 * NT + t : 0 * NT + t + 1],
                s1=cw_s[:, 2 * NT + t : 2 * NT + t + 1],
            )
            v._custom_dve(
                blend2,
                out=SB[:, ts],
                in0=g[:, 256:512],
                in1=g[:, 768:1024],
                s0=cw_s[:, 1 * NT + t : 1 * NT + t + 1],
                s1=cw_s[:, 3 * NT + t : 3 * NT + t + 1],
            )

        # ---- plan-B slots: one batched T+B add for the tap ----
        if NB:
            cr = bp.tile([128, NB, 256], BF16, name="cr")
            v.tensor_tensor(
                cr[:], ST[:, A_SLOTS:TPC], SB[:, A_SLOTS:TPC], AluOp.add
            )

        # ---- transposes into PSUM, copies into rhsT ----
        rhsT = rp.tile([128, 2, SLOC], BF16, name="rhsT")
        pts = []
        for ts in range(TPC):
            tsl = ts % 4
            if tsl == 0:
                pt = tp.tile([128, 2, 4, 128], BF16, name="tpsum", space="PSUM")
                pts.append(pt)
            pt = pts[-1]
            for ch in range(2):
                csl = slice(ch * 128, (ch + 1) * 128)
                if ts < A_SLOTS:
                    nc.tensor.matmul(
                        pt[:, ch, tsl, :], ST[:, ts, csl], id_s[:],
                        is_transpose=True, start=True, stop=False,
                    )
                    nc.tensor.matmul(
                        pt[:, ch, tsl, :], SB[:, ts, csl], id_s[:],
                        is_transpose=True, start=False, stop=True,
                    )
                else:
                    nc.tensor.matmul(
                        pt[:, ch, tsl, :], cr[:, ts - A_SLOTS, csl], id_s[:],
                        is_transpose=True, start=True, stop=True,
                    )
            if tsl == 3:
                ts0 = ts - 3
                nc.scalar.copy(
                    rhsT[:, :, ts0 * 128 : (ts0 + 4) * 128].rearrange(
                        "p a (c b) -> p a c b", c=4
                    ),
                    pt[:],
                )

        # ---- main matmuls for this tap ----
        for h in range(2):
            for ch in range(2):
                j = 2 * k + ch
                for sh in range(2):
                    nc.tensor.matmul(
                        ps[h][:, sh * 512 : (sh + 1) * 512],
                        wT_s[:, j, h * 128 : (h + 1) * 128],
                        rhsT[:, ch, sh * 512 : (sh + 1) * 512],
                        start=(j == 0),
                        stop=(j == 17),
                    )
            if k == K - 1:
                # stream each output half out as soon as its PSUM region
                # finishes, overlapping the other half's matmuls
                nc.scalar.copy(osb[:, h, :], ps[h][:])
                nc.sync.dma_start(out[:, h, :], osb[:, h, :])

    ctx.close()


# ---------------- host-side prep ----------------

def _host_maps(offset):
    """offset [N, 18, 64, 64] f32 -> (ridx [N,K,S] i32, cw [N,4,K,S] f32).

    Mirrors the reference bilinear semantics: corners floor/floor+1 with
    zero weight outside [0, 63]; gather window start clipped to [0, 62].
    """
    off = np.asarray(offset, np.float32).reshape(N, K, 2, S)
    ky, kx = np.meshgrid(np.arange(KH), np.arange(KW), indexing="ij")
    ky = ky.reshape(K, 1).astype(np.float32)
    kx = kx.reshape(K, 1).astype(np.float32)
    ho, wo = np.meshgrid(np.arange(H), np.arange(W), indexing="ij")
    base_y = (ho.reshape(1, S) - 1 + ky).astype(np.float32)  # [K, S]
    base_x = (wo.reshape(1, S) - 1 + kx).astype(np.float32)

    def axis_weights(p):
        """p [N,K,S] coords -> (c, w0, w1): window start + slot weights."""
        f = np.floor(p)
        l = p - f
        h = 1.0 - l
        v0 = (f >= 0) & (f <= 63)
        v1 = (f + 1 >= 0) & (f + 1 <= 63)
        c = np.clip(f, 0, 62)
        w0 = (c == f) * h * v0 + (c == f + 1) * l * v1
        w1 = (c + 1 == f) * h * v0 + (c + 1 == f + 1) * l * v1
        return c, w0.astype(np.float32), w1.astype(np.float32)

    py = base_y[None] + off[:, :, 0]   # [N, K, S]
    px = base_x[None] + off[:, :, 1]
    cy, wy0, wy1 = axis_weights(py)
    cx, wx0, wx1 = axis_weights(px)
    ridx = (cy * 64 + cx).astype(np.int32)
    cw = np.stack([wy0 * wx0, wy1 * wx0, wy0 * wx1, wy1 * wx1], axis=1)
    return ridx, cw


def core_inputs(x, offset, weight):
    """Full inputs (np f32) -> list of 8 per-core input dicts."""
    bf = ml_dtypes.bfloat16
    x = np.asarray(x, np.float32)
    weight = np.asarray(weight, np.float32)

    # y-pair-interleaved channels-last images, bf16: xi[r] = [x[r], x[r+64]]
    xis = []
    for n in range(N):
        xcl = np.ascontiguousarray(x[n].reshape(CIN, S).T)  # [4096, 256]
        xi = np.zeros((S + 64, 2 * CIN), np.float32)  # 64 pad rows: overlapped AP
        xi[:S, :CIN] = xcl
        xi[: S - W, CIN:] = xcl[W:]
        xis.append(xi.astype(bf))

    # lhsT [k*256+c, o]
    wk = weight.reshape(COUT, CIN, K)           # [o, c, k]
    wT = np.ascontiguousarray(wk.transpose(2, 1, 0).reshape(K * CIN, COUT)).astype(bf)

    ident = np.eye(128, dtype=bf)

    ridx, cw = _host_maps(offset)               # [N,K,S] i32, [N,4,K,S] f32

    ins = []
    for core in range(8):
        n, qtr = core // 4, core % 4
        sl = slice(qtr * SLOC, (qtr + 1) * SLOC)

        def lay(a):  # [K, S] -> [128, NT]: [p, k*8+ts] = a[k, p*8+ts]
            aq = a[:, sl].reshape(K, 128, TPC)      # [k, p, ts]
            return np.ascontiguousarray(aq.transpose(1, 0, 2).reshape(128, NT))

        cwc = np.concatenate(
            [lay(cw[n, a]) for a in range(4)], axis=1
        ).astype(np.float32)                     # [128, 4*NT]

        # dma_gather index table: flat sample i = ts*128 + p reads row
        # rl[p, k*8+ts]; wrapped [16, 64] (i = c*16 + r), replicated
        # across the 8 gpsimd core groups.
        rl = lay(ridx[n])                        # [128, NT] == [p, k*8+ts]
        idx16 = np.zeros((128, K * 64), np.int16)
        for k in range(K):
            flat = rl[:, k * TPC : (k + 1) * TPC].T.reshape(SLOC)  # i = ts*128+p
            wrapped = flat.reshape(64, 16).T.astype(np.int16)      # [16, 64]
            idx16[:, k * 64 : (k + 1) * 64] = np.tile(wrapped, (8, 1))

        ins.append({
            "xi": xis[n],
            "wT": wT,
            "ident": ident,
            "ridx": rl.astype(np.int32),
            "idx16": idx16,
            "cw": np.ascontiguousarray(cwc),
        })
    return ins


def assemble(results):
    """list of 8 per-core {'out': [128,2,1024] bf16} -> [2,256,64,64] f32."""
    out = np.zeros((N, COUT, S), np.float32)
    for core in range(8):
        n, qtr = core // 4, core % 4
        o = np.asarray(results[core]["out"]).astype(np.float32)
        o = o.transpose(1, 0, 2).reshape(COUT, SLOC)  # [o, s'] s' = ts*128+p
        o = o.reshape(COUT, TPC, 128).transpose(0, 2, 1).reshape(COUT, SLOC)
        out[n, :, qtr * SLOC : (qtr + 1) * SLOC] = o
    return out.reshape(N, COUT, H, W)


def declare_io(nc):
    ins = {
        "xi": nc.dram_tensor("xi", [S + 64, 2 * CIN], BF16, kind="ExternalInput").ap(),
        "wT": nc.dram_tensor("wT", [K * CIN, COUT], BF16, kind="ExternalInput").ap(),
        "ident": nc.dram_tensor("ident", [128, 128], BF16, kind="ExternalInput").ap(),
        "ridx": nc.dram_tensor("ridx", [128, NT], I32, kind="ExternalInput").ap(),
        "idx16": nc.dram_tensor("idx16", [128, K * 64], I16, kind="ExternalInput").ap(),
        "cw": nc.dram_tensor("cw", [128, 4 * NT], F32, kind="ExternalInput").ap(),
    }
    outs = {
        "out": nc.dram_tensor("out", [128, 2, SLOC], BF16, kind="ExternalOutput").ap(),
    }
    return outs, ins


def build_module():
    from concourse import bacc

    nc = bacc.Bacc("TRN2", target_bir_lowering=False, debug=False, num_devices=8)
    outs, ins = declare_io(nc)
    with tile.TileContext(nc) as tc:
        build_core_kernel(nc, tc, outs, ins)
    nc.compile()
    return nc


_NC_CACHE = []


def kernel(x, offset, weight):
    """Full (unsharded) inputs -> full output, computed on 8 NeuronCores."""
    import time

    from concourse.bass_utils import run_bass_kernel_spmd

    if not _NC_CACHE:
        _NC_CACHE.append(build_module())
    nc = _NC_CACHE[0]
    core_ins = core_inputs(x, offset, weight)
    last = None
    for attempt in range(3):
        try:
            res = run_bass_kernel_spmd(nc, core_ins, core_ids=list(range(8)))
            return assemble(res.results)
        except Exception as e:  # transient device-session failures
            last = e
            time.sleep(2.0 * (attempt + 1))
    raise last


# revision 32
# speedup vs baseline: 1.2700x; 1.2700x over previous
"""Self-contained Trainium2 Bass kernel for deformable conv 2d.

kernel(x, offset, weight) -> out, matching the jax reference:
  x[2,256,64,64] f32, offset[2,18,64,64] f32, weight[256,256,3,3] f32
  -> out[2,256,64,64] f32 (KH=KW=3, stride=1, pad=1, dil=1, DG=1).

Runs SPMD on 8 NeuronCores, data-parallel: core = (batch, spatial quarter).

v3 design (vs the v1 baseline):
  - bilinear sample indices + corner weights precomputed on HOST (numpy);
    the device does only: batched indirect gathers, fused DVE blends,
    PE transposes, and the main matmuls.
  - gathers batched 4 slots (512 descriptors) per instruction to amortize
    the ~1us SWDGE fixed overhead per DMA instruction.
  - blend uses a custom fused DVE op: out = in0*s0 + in1*s1 (per-partition
    scalars), two ops per 128-sample slot (top row pair / bottom row pair).
  - the final top+bottom add either runs as one batched DVE tensor_tensor
    per tap (plan B slots) or rides the PE transpose via PSUM accumulation
    (plan A slots); A_SLOTS tunes the DVE/PE balance.
  - output stored bf16 (halves the output write traffic).
"""

import sys

for _p in ("/opt/trn_rl_repo",):
    if _p not in sys.path:
        sys.path.insert(0, _p)


import numpy as np
import ml_dtypes

import concourse.bass as bass
import concourse.mybir as mybir
import concourse.tile as tile

F32 = mybir.dt.float32
BF16 = mybir.dt.bfloat16
I32 = mybir.dt.int32
I16 = mybir.dt.int16

N, CIN, H, W = 2, 256, 64, 64
COUT = 256
KH = KW = 3
K = KH * KW
S = H * W            # 4096 output positions per batch
SLOC = S // 4        # 1024 per core
TPC = 8              # ts slots per tap (SLOC/128)
NT = K * TPC         # 72 slots of [128 samples]

A_SLOTS = 0          # slots per tap whose T+B add rides the PE transpose
                     # (PE matmul PSUM writes must be is_transpose to hit
                     # sub-bank offsets; accumulate variant pending test)
GATHER_MODE = "half"  # "slot": indirect_dma_start per slot (128 desc each)
                      # "tap": dma_gather per tap (1024 idx)
                      # "half": dma_gather per half tap (512 idx) — best
                      # "tap2": dma_gather per 2 taps (2048 idx) — wedges HW

AluOp = mybir.AluOpType


# ---------------- custom fused DVE op ----------------

_BLEND2 = []


def _get_blend2():
    """Register (once) and return the fused op out = in0*s0 + in1*s1."""
    if _BLEND2:
        return _BLEND2[0]
    import concourse.dve_ops as dve_ops_mod
    from concourse.dve_ops import DveOp
    from concourse.dve_spec import Spec, Src0, Src1, C0, C1
    from concourse.dve_spec import lower as dve_lower
    from concourse.dve_uop import DveOpSpec

    name = "BLEND2_DEFORM_ANT"
    if name not in dve_ops_mod._SUB_OPCODE_FOR_NAME:
        spec = Spec(body=Src0 * C0 + Src1 * C1)
        row = max(dve_ops_mod._SUB_OPCODE_FOR_NAME.values()) + 1
        assert row < 0x20
        shas = {}
        for ver in ("v3", "v4"):
            dspec = DveOpSpec(
                name=name, opcode=row, uops=dve_lower(spec, ver=ver), rd1_en=True
            )
            shas[ver] = dspec.sha(ver)
        op = DveOp(name, spec, subdim=False, uops_sha=shas)
        dve_ops_mod._SUB_OPCODE_FOR_NAME[name] = row
        dve_ops_mod.OPS.append(op)
        dve_ops_mod.CUSTOM_DVE_SPECS[name] = spec
    _BLEND2.append(
        next(op for op in dve_ops_mod.OPS if op.name == name)
    )
    return _BLEND2[0]


def build_core_kernel(nc, tc, outs, ins):
    """Emit the per-core kernel. ins/outs are dicts of DRAM APs."""
    from contextlib import ExitStack

    from concourse.ap import AP

    xi = ins["xi"]          # [4096+64, 512] bf16 y-pair-interleaved image
    wT = ins["wT"]          # [2304, 256] bf16 lhsT
    ridx = ins["ridx"]      # [128, NT] i32 gather row per (partition, slot)
    idx16 = ins["idx16"]    # [128, K*64] i16 wrapped gather indices
    cw = ins["cw"]          # [128, 4*NT] f32 corner weights (T0,B0,T1,B1)
    ident = ins["ident"]    # [128, 128] bf16 identity
    out = outs["out"]       # [128, 2, 1024] bf16

    # overlapped view of xi: row r covers xi rows r and r+1 (the 4 corners)
    xi_ov = AP(xi.tensor, 0, [[2 * CIN, S], [1, 4 * CIN]])

    blend2 = _get_blend2()

    ctx = ExitStack()
    sp = ctx.enter_context(tc.tile_pool(name="static", bufs=1))
    gp = ctx.enter_context(tc.tile_pool(name="gather", bufs=3))
    bp = ctx.enter_context(tc.tile_pool(name="blend", bufs=4))
    rp = ctx.enter_context(tc.tile_pool(name="rhsT", bufs=3))
    pp = ctx.enter_context(tc.tile_pool(name="psum", bufs=1, space="PSUM"))
    tp = ctx.enter_context(tc.tile_pool(name="tpsum", bufs=2, space="PSUM"))

    v = nc.vector

    # ---- static loads (indices first: they gate the gathers) ----
    ridx_s = sp.tile([128, NT], I32, name="ridx_s")
    nc.sync.dma_start(ridx_s[:], ridx)
    idx_s = sp.tile([128, K, 64], I16, name="idx_s")
    nc.sync.dma_start(idx_s[:], idx16.rearrange("p (k c) -> p k c", k=K))
    cw_s = sp.tile([128, 4 * NT], F32, name="cw_s")
    nc.sync.dma_start(cw_s[:], cw)
    wT_s = sp.tile([128, 18, 256], BF16, name="wT_s")
    nc.sync.dma_start(wT_s[:], wT.rearrange("(j p) o -> p j o", p=128))
    id_s = sp.tile([128, 128], BF16, name="id_s")
    nc.sync.dma_start(id_s[:], ident)

    ps = [pp.tile([128, SLOC], F32, name=f"psum{h}") for h in range(2)]
    osb = sp.tile([128, 2, SLOC], BF16, name="osb")

    NB = TPC - A_SLOTS   # plan-B slots per tap (batched DVE add)
    g2_hold = None

    for k in range(K):
        # ---- gathers, one of three mechanisms ----
        if GATHER_MODE == "slot":
            gts = [gp.tile([128, 1024], BF16, name=f"g{ts}") for ts in range(TPC)]
            for ts in range(TPC):
                t = k * TPC + ts
                nc.gpsimd.indirect_dma_start(
                    out=gts[ts][:],
                    out_offset=None,
                    in_=xi,
                    in_offset=bass.IndirectOffsetOnAxis(
                        ap=ridx_s[:, t : t + 1], axis=0
                    ),
                )
        elif GATHER_MODE == "tap":
            g = gp.tile([128, TPC, 1024], BF16, name="g")
            nc.gpsimd.dma_gather(
                out_ap=g[:],
                in_ap=xi_ov,
                idxs_ap=idx_s[:, k],
                num_idxs=SLOC,
                num_idxs_reg=SLOC,
                elem_size=4 * CIN,
                elem_step=2 * CIN,
            )
            gts = [g[:, ts] for ts in range(TPC)]
        elif GATHER_MODE == "half":
            g = gp.tile([128, TPC, 1024], BF16, name="g")
            for hf in range(2):
                nc.gpsimd.dma_gather(
                    out_ap=g[:, hf * 4 : (hf + 1) * 4],
                    in_ap=xi_ov,
                    idxs_ap=idx_s[:, k, hf * 32 : (hf + 1) * 32],
                    num_idxs=SLOC // 2,
                    num_idxs_reg=SLOC // 2,
                    elem_size=4 * CIN,
                    elem_step=2 * CIN,
                )
            gts = [g[:, ts] for ts in range(TPC)]
        elif GATHER_MODE == "tap2":
            if k % 2 == 0:
                ntap = 2 if k + 1 < K else 1
                g2 = gp.tile([128, ntap * TPC, 1024], BF16, name="g2")
                nc.gpsimd.dma_gather(
                    out_ap=g2[:],
                    in_ap=xi_ov,
                    idxs_ap=idx_s[:, k : k + ntap].rearrange("p a c -> p (a c)"),
                    num_idxs=ntap * SLOC,
                    num_idxs_reg=ntap * SLOC,
                    elem_size=4 * CIN,
                    elem_step=2 * CIN,
                )
                g2_hold = g2
            base = (k % 2) * TPC
            gts = [g2_hold[:, base + ts] for ts in range(TPC)]
        else:
            raise ValueError(GATHER_MODE)

        # ---- fused blends: ST = T0*cw0 + T1*cw2, SB = B0*cw1 + B1*cw3 ----
        ST = bp.tile([128, TPC, 256], BF16, name="ST")
        SB = bp.tile([128, TPC, 256], BF16, name="SB")
        for ts in range(TPC):
            t = k * TPC + ts
            g = gts[ts]
            v._custom_dve(
                blend2,
                out=ST[:, ts],
                in0=g[:, 0:256],
                in1=g[:, 512:768],
                s0=cw_s[:, 0 * NT + t : 0 * NT + t + 1],
                s1=cw_s[:, 2 * NT + t : 2 * NT + t + 1],
            )
            v._custom_dve(
                blend2,
                out=SB[:, ts],
                in0=g[:, 256:512],
                in1=g[:, 768:1024],
                s0=cw_s[:, 1 * NT + t : 1 * NT + t + 1],
                s1=cw_s[:, 3 * NT + t : 3 * NT + t + 1],
            )

        # ---- plan-B slots: one batched T+B add for the tap ----
        if NB:
            cr = bp.tile([128, NB, 256], BF16, name="cr")
            v.tensor_tensor(
                cr[:], ST[:, A_SLOTS:TPC], SB[:, A_SLOTS:TPC], AluOp.add
            )

        # ---- transposes into PSUM, copies into rhsT ----
        rhsT = rp.tile([128, 2, SLOC], BF16, name="rhsT")
        pts = []
        for ts in range(TPC):
            tsl = ts % 4
            if tsl == 0:
                pt = tp.tile([128, 2, 4, 128], BF16, name="tpsum", space="PSUM")
                pts.append(pt)
            pt = pts[-1]
            for ch in range(2):
                csl = slice(ch * 128, (ch + 1) * 128)
                if ts < A_SLOTS:
                    nc.tensor.matmul(
                        pt[:, ch, tsl, :], ST[:, ts, csl], id_s[:],
                        is_transpose=True, start=True, stop=False,
                    )
                    nc.tensor.matmul(
                        pt[:, ch, tsl, :], SB[:, ts, csl], id_s[:],
                        is_transpose=True, start=False, stop=True,
                    )
                else:
                    nc.tensor.matmul(
                        pt[:, ch, tsl, :], cr[:, ts - A_SLOTS, csl], id_s[:],
                        is_transpose=True, start=True, stop=True,
                    )
            if tsl == 3:
                ts0 = ts - 3
                nc.scalar.copy(
                    rhsT[:, :, ts0 * 128 : (ts0 + 4) * 128].rearrange(
                        "p a (c b) -> p a c b", c=4
                    ),
                    pt[:],
                )

        # ---- main matmuls for this tap ----
        for h in range(2):
            for ch in range(2):
                j = 2 * k + ch
                for sh in range(2):
                    nc.tensor.matmul(
                        ps[h][:, sh * 512 : (sh + 1) * 512],
                        wT_s[:, j, h * 128 : (h + 1) * 128],
                        rhsT[:, ch, sh * 512 : (sh + 1) * 512],
                        start=(j == 0),
                        stop=(j == 17),
                    )
            if k == K - 1:
                # stream each output half out as soon as its PSUM region
                # finishes, overlapping the other half's matmuls
                nc.scalar.copy(osb[:, h, :], ps[h][:])
                nc.sync.dma_start(out[:, h, :], osb[:, h, :])

    ctx.close()


# ---------------- host-side prep ----------------

def _host_maps(offset):
    """offset [N, 18, 64, 64] f32 -> (ridx [N,K,S] i32, cw [N,4,K,S] f32).

    Mirrors the reference bilinear semantics: corners floor/floor+1 with
    zero weight outside [0, 63]; gather window start clipped to [0, 62].
    """
    off = np.asarray(offset, np.float32).reshape(N, K, 2, S)
    ky, kx = np.meshgrid(np.arange(KH), np.arange(KW), indexing="ij")
    ky = ky.reshape(K, 1).astype(np.float32)
    kx = kx.reshape(K, 1).astype(np.float32)
    ho, wo = np.meshgrid(np.arange(H), np.arange(W), indexing="ij")
    base_y = (ho.reshape(1, S) - 1 + ky).astype(np.float32)  # [K, S]
    base_x = (wo.reshape(1, S) - 1 + kx).astype(np.float32)

    def axis_weights(p):
        """p [N,K,S] coords -> (c, w0, w1): window start + slot weights."""
        f = np.floor(p)
        l = p - f
        h = 1.0 - l
        v0 = (f >= 0) & (f <= 63)
        v1 = (f + 1 >= 0) & (f + 1 <= 63)
        c = np.clip(f, 0, 62)
        w0 = (c == f) * h * v0 + (c == f + 1) * l * v1
        w1 = (c + 1 == f) * h * v0 + (c + 1 == f + 1) * l * v1
        return c, w0.astype(np.float32), w1.astype(np.float32)

    py = base_y[None] + off[:, :, 0]   # [N, K, S]
    px = base_x[None] + off[:, :, 1]
    cy, wy0, wy1 = axis_weights(py)
    cx, wx0, wx1 = axis_weights(px)
    ridx = (cy * 64 + cx).astype(np.int32)
    cw = np.stack([wy0 * wx0, wy1 * wx0, wy0 * wx1, wy1 * wx1], axis=1)
    return ridx, cw


def core_inputs(x, offset, weight):
    """Full inputs (np f32) -> list of 8 per-core input dicts."""
    bf = ml_dtypes.bfloat16
    x = np.asarray(x, np.float32)
    weight = np.asarray(weight, np.float32)

    # y-pair-interleaved channels-last images, bf16: xi[r] = [x[r], x[r+64]]
    xis = []
    for n in range(N):
        xcl = np.ascontiguousarray(x[n].reshape(CIN, S).T)  # [4096, 256]
        xi = np.zeros((S + 64, 2 * CIN), np.float32)  # 64 pad rows: overlapped AP
        xi[:S, :CIN] = xcl
        xi[: S - W, CIN:] = xcl[W:]
        xis.append(xi.astype(bf))

    # lhsT [k*256+c, o]
    wk = weight.reshape(COUT, CIN, K)           # [o, c, k]
    wT = np.ascontiguousarray(wk.transpose(2, 1, 0).reshape(K * CIN, COUT)).astype(bf)

    ident = np.eye(128, dtype=bf)

    ridx, cw = _host_maps(offset)               # [N,K,S] i32, [N,4,K,S] f32

    ins = []
    for core in range(8):
        n, qtr = core // 4, core % 4
        sl = slice(qtr * SLOC, (qtr + 1) * SLOC)

        def lay(a):  # [K, S] -> [128, NT]: [p, k*8+ts] = a[k, p*8+ts]
            aq = a[:, sl].reshape(K, 128, TPC)      # [k, p, ts]
            return np.ascontiguousarray(aq.transpose(1, 0, 2).reshape(128, NT))

        cwc = np.concatenate(
            [lay(cw[n, a]) for a in range(4)], axis=1
        ).astype(np.float32)                     # [128, 4*NT]

        # dma_gather index table: flat sample i = ts*128 + p reads row
        # rl[p, k*8+ts]; wrapped [16, 64] (i = c*16 + r), replicated
        # across the 8 gpsimd core groups.
        rl = lay(ridx[n])                        # [128, NT] == [p, k*8+ts]
        idx16 = np.zeros((128, K * 64), np.int16)
        for k in range(K):
            flat = rl[:, k * TPC : (k + 1) * TPC].T.reshape(SLOC)  # i = ts*128+p
            wrapped = flat.reshape(64, 16).T.astype(np.int16)      # [16, 64]
            idx16[:, k * 64 : (k + 1) * 64] = np.tile(wrapped, (8, 1))

        ins.append({
            "xi": xis[n],
            "wT": wT,
            "ident": ident,
            "ridx": rl.astype(np.int32),
            "idx16": idx16,
            "cw": np.ascontiguousarray(cwc),
        })
    return ins


def assemble(results):
    """list of 8 per-core {'out': [128,2,1024] bf16} -> [2,256,64,64] f32."""
    out = np.zeros((N, COUT, S), np.float32)
    for core in range(8):
        n, qtr = core // 4, core % 4
        o = np.asarray(results[core]["out"]).astype(np.float32)
        o = o.transpose(1, 0, 2).reshape(COUT, SLOC)  # [o, s'] s' = ts*128+p
        o = o.reshape(COUT, TPC, 128).transpose(0, 2, 1).reshape(COUT, SLOC)
        out[n, :, qtr * SLOC : (qtr + 1) * SLOC] = o
    return out.reshape(N, COUT, H, W)


def declare_io(nc):
    ins = {
        "xi": nc.dram_tensor("xi", [S + 64, 2 * CIN], BF16, kind="ExternalInput").ap(),
        "wT": nc.dram_tensor("wT", [K * CIN, COUT], BF16, kind="ExternalInput").ap(),
        "ident": nc.dram_tensor("ident", [128, 128], BF16, kind="ExternalInput").ap(),
        "ridx": nc.dram_tensor("ridx", [128, NT], I32, kind="ExternalInput").ap(),
        "idx16": nc.dram_tensor("idx16", [128, K * 64], I16, kind="ExternalInput").ap(),
        "cw": nc.dram_tensor("cw", [128, 4 * NT], F32, kind="ExternalInput").ap(),
    }
    outs = {
        "out": nc.dram_tensor("out", [128, 2, SLOC], BF16, kind="ExternalOutput").ap(),
    }
    return outs, ins


def build_module():
    from concourse import bacc

    nc = bacc.Bacc("TRN2", target_bir_lowering=False, debug=False, num_devices=8)
    outs, ins = declare_io(nc)
    with tile.TileContext(nc) as tc:
        build_core_kernel(nc, tc, outs, ins)
    nc.compile()
    return nc


_NC_CACHE = []


def kernel(x, offset, weight):
    """Full (unsharded) inputs -> full output, computed on 8 NeuronCores."""
    import time

    from concourse.bass_utils import run_bass_kernel_spmd

    if not _NC_CACHE:
        _NC_CACHE.append(build_module())
    nc = _NC_CACHE[0]
    core_ins = core_inputs(x, offset, weight)
    last = None
    for attempt in range(3):
        try:
            res = run_bass_kernel_spmd(nc, core_ins, core_ids=list(range(8)))
            return assemble(res.results)
        except Exception as e:  # transient device-session failures
            last = e
            time.sleep(2.0 * (attempt + 1))
    raise last
